# revision 2
# baseline (speedup 1.0000x reference)
"""Trainium2 Bass kernel for nn_MultiHeadAttention_19396072309379.

Module math (per reference): all H=8 heads identical; V projected from `key`;
causal mask; softmax; concat of identical heads @ Wo  ==  o @ (sum of Wo row
blocks).  Computed as single-head attention with a reduced Wo.

Sharding: 8 cores = 4 batches x 2 "parity" halves.  Each core owns 8 of the 16
query blocks (128 rows each) of one batch, paired {i, 15-i} so causal work is
balanced (68 block-pairs per core).  Both parities run the SAME program: the
attention loop uses unified per-key-block suffix widths (max over parities) and
a per-core mask input resolves the diagonal/extra-block difference as data.

On-chip layout is fully transposed ("T" = [feature, seq]): scoresT[ks, qs] =
kT_proj_blk.T @ qT_proj; exp via ACT (scale=1/sqrt(DK) fused); P stays
transposed so PV needs no P transpose: oT = v1.T @ expT where v1 = [v | 1] --
the ones column accumulates the softmax denominators for free (row 64).
Normalization is folded in after a K=1 broadcast matmul of the sums row.
All matmuls run fp32r (full-rate fp32 mode, ~1.6e-4 rel err).
"""

import numpy as np

B, S, D, H, DK, DV = 4, 2048, 512, 8, 64, 64
NB = S // 128  # 16 key/query blocks per batch
QB = 8  # query blocks per core
SQ = QB * 128  # 1024 query rows per core
N_CORES = 8

# per-parity query block sets (pairs {i, 15-i} -> equal causal work 68)
BLOCKS = {
    0: [0, 2, 4, 6, 9, 11, 13, 15],
    1: [1, 3, 5, 7, 8, 10, 12, 14],
}
# unified suffix width (in 128-blocks) for key-block j = max over parities of
# count of local query blocks with global index >= j
WIDTHS = [
    max(sum(1 for g in BLOCKS[p] if g >= j) for p in (0, 1)) for j in range(NB)
]


def _build():
    import concourse.mybir as mybir
    import concourse.tile as tile
    from concourse import bacc

    F32 = mybir.dt.float32
    F32R = mybir.dt.float32r
    AF = mybir.ActivationFunctionType

    nc = bacc.Bacc("TRN2", target_bir_lowering=False, debug=False, num_devices=N_CORES)
    d_qT = nc.dram_tensor("qT", [D, SQ], F32, kind="ExternalInput").ap()
    d_kT = nc.dram_tensor("kT", [D, S], F32, kind="ExternalInput").ap()
    d_wq = nc.dram_tensor("wq", [D, DK], F32, kind="ExternalInput").ap()
    d_wk = nc.dram_tensor("wk", [D, DK], F32, kind="ExternalInput").ap()
    d_wv = nc.dram_tensor("wv", [D, DV], F32, kind="ExternalInput").ap()
    d_wo = nc.dram_tensor("wo", [DV, D], F32, kind="ExternalInput").ap()
    d_bq = nc.dram_tensor("bq", [DK, 1], F32, kind="ExternalInput").ap()
    d_bk = nc.dram_tensor("bk", [DK, 1], F32, kind="ExternalInput").ap()
    d_bm = nc.dram_tensor("bm", [NB, 128, 128], F32, kind="ExternalInput").ap()
    d_ones = nc.dram_tensor("ones", [128, 65], F32, kind="ExternalInput").ap()
    d_id = nc.dram_tensor("ident", [DV, DV], F32, kind="ExternalInput").ap()
    d_out = nc.dram_tensor("out", [SQ, D], F32, kind="ExternalOutput").ap()

    with (
        tile.TileContext(nc) as tc,
        nc.allow_low_precision(reason="fp32r attention kernel"),
    ):
        with (
            tc.tile_pool(name="const", bufs=1) as cpool,
            tc.tile_pool(name="acts", bufs=1) as apool,
            tc.tile_pool(name="work", bufs=3) as wpool,
        ):
            # ---- constants ----
            wq_t = cpool.tile([128, 4, DK], F32R)
            nc.sync.dma_start(
                wq_t[:], d_wq.rearrange("(c p) k -> p c k", p=128).bitcast(F32R)
            )
            wk_t = cpool.tile([128, 4, DK], F32R)
            nc.sync.dma_start(
                wk_t[:], d_wk.rearrange("(c p) k -> p c k", p=128).bitcast(F32R)
            )
            wv_t = cpool.tile([128, 4, DV], F32R)
            nc.sync.dma_start(
                wv_t[:], d_wv.rearrange("(c p) k -> p c k", p=128).bitcast(F32R)
            )
            wo_t = cpool.tile([DV, D], F32R)
            nc.sync.dma_start(wo_t[:], d_wo.bitcast(F32R))
            bq_t = cpool.tile([DK, 1], F32)
            nc.sync.dma_start(bq_t[:], d_bq[:])
            bk_t = cpool.tile([DK, 1], F32)
            nc.sync.dma_start(bk_t[:], d_bk[:])
            ones_t = cpool.tile([128, 65], F32R)
            nc.sync.dma_start(ones_t[:], d_ones.bitcast(F32R))
            id_t = cpool.tile([DV, DV], F32)
            nc.sync.dma_start(id_t[:], d_id[:])
            bm_t = cpool.tile([128, NB, 128], F32R)
            nc.sync.dma_start(
                bm_t[:], d_bm.rearrange("j p m -> p j m").bitcast(F32R)
            )

            # ---- activations (pre-transposed on host) ----
            qT_act = apool.tile([128, 4, SQ], F32R)
            qT_r = d_qT.rearrange("(c p) s -> p c s", p=128).bitcast(F32R)
            for c in range(4):
                nc.sync.dma_start(qT_act[:, c, :], qT_r[:, c, :])
            kT_act = apool.tile([128, 4, S], F32R)
            kT_r = d_kT.rearrange("(c p) s -> p c s", p=128).bitcast(F32R)
            for c in range(4):
                for h in range(2):
                    nc.sync.dma_start(
                        kT_act[:, c, 1024 * h : 1024 * (h + 1)],
                        kT_r[:, c, 1024 * h : 1024 * (h + 1)],
                    )

            # ---- projections: xT_proj[dk, s] = W.T @ xT ----
            pp = tc.alloc_tile_pool(name="pproj", bufs=2, space="PSUM")
            qT_proj = apool.tile([DK, SQ], F32R)
            for n in range(SQ // 512):
                ps = pp.tile([DK, 512], F32, tag="psproj")
                for c in range(4):
                    nc.tensor.matmul(
                        ps[:],
                        wq_t[:, c, :],
                        qT_act[:, c, 512 * n : 512 * (n + 1)],
                        start=(c == 0),
                        stop=(c == 3),
                    )
                nc.scalar.activation(
                    qT_proj[:, 512 * n : 512 * (n + 1)], ps[:], AF.Identity,
                    bias=bq_t[:], scale=1.0,
                )
            kT_proj = apool.tile([DK, S], F32R)
            for n in range(S // 512):
                ps = pp.tile([DK, 512], F32, tag="psproj")
                for c in range(4):
                    nc.tensor.matmul(
                        ps[:],
                        wk_t[:, c, :],
                        kT_act[:, c, 512 * n : 512 * (n + 1)],
                        start=(c == 0),
                        stop=(c == 3),
                    )
                nc.scalar.activation(
                    kT_proj[:, 512 * n : 512 * (n + 1)], ps[:], AF.Identity,
                    bias=bk_t[:], scale=1.0,
                )
            vT_proj = apool.tile([DV, S], F32)
            for n in range(S // 512):
                ps = pp.tile([DV, 512], F32, tag="psproj")
                for c in range(4):
                    nc.tensor.matmul(
                        ps[:],
                        wv_t[:, c, :],
                        kT_act[:, c, 512 * n : 512 * (n + 1)],
                        start=(c == 0),
                        stop=(c == 3),
                    )
                nc.scalar.copy(vT_proj[:, 512 * n : 512 * (n + 1)], ps[:])

            # ---- v1 = [v | 1]: transpose vT 128-col blocks via PE ----
            v1 = apool.tile([128, NB, DV + 1], F32R)
            for j in range(NB):
                pt = pp.tile([128, DV], F32, tag="pstrans")
                nc.tensor.transpose(
                    pt[:], vT_proj[:, 128 * j : 128 * (j + 1)], id_t[:]
                )
                nc.vector.tensor_copy(v1[:, j, 0:DV], pt[:])
                nc.vector.tensor_copy(v1[:, j, DV : DV + 1], ones_t[:, 0:1])

            pp.release()
            # ---- attention: scoresT -> exp -> mask -> PV (accumulated) ----
            pacc = tc.alloc_tile_pool(name="pacc", bufs=1, space="PSUM")
            psc = tc.alloc_tile_pool(name="pscore", bufs=3, space="PSUM")
            po = pacc.tile([DV + 1, SQ], F32)
            for j in range(NB):
                wblk = WIDTHS[j]
                c0 = 128 * (QB - wblk)
                cols = 128 * wblk
                expT = wpool.tile([128, 1024], F32R, tag="expT")
                nchunks = (cols + 511) // 512
                for n in range(nchunks):
                    nsz = min(512, cols - 512 * n)
                    ps_s = psc.tile([128, 512], F32, tag="pss")
                    nc.tensor.matmul(
                        ps_s[:, 0:nsz],
                        kT_proj[:, 128 * j : 128 * (j + 1)],
                        qT_proj[:, c0 + 512 * n : c0 + 512 * n + nsz],
                        start=True,
                        stop=True,
                    )
                    nc.scalar.activation(
                        expT[:, 512 * n : 512 * n + nsz], ps_s[:, 0:nsz],
                        AF.Exp, bias=0.0, scale=0.125,
                    )
                # mask the first block-column (diag / extra / valid as data)
                nc.vector.tensor_mul(
                    expT[:, 0:128], expT[:, 0:128], bm_t[:, j, :]
                )
                for n in range(nchunks):
                    nsz = min(512, cols - 512 * n)
                    nc.tensor.matmul(
                        po[:, c0 + 512 * n : c0 + 512 * n + nsz],
                        v1[:, j, :],
                        expT[:, 512 * n : 512 * n + nsz],
                        start=(j == 0),
                        stop=(j == NB - 1),
                        skip_group_check=True,
                    )

            psc.release()
            # ---- epilogue: normalize by sums row, output projection ----
            oT_s = apool.tile([DV, SQ], F32R)
            nc.scalar.copy(oT_s[:], po[0:DV, :])
            srow = apool.tile([1, SQ], F32R)
            nc.scalar.copy(srow[:], po[DV : DV + 1, :])
            pb = pacc.tile([DV, SQ], F32)
            for n in range(SQ // 512):
                nc.tensor.matmul(
                    pb[:, 512 * n : 512 * (n + 1)],
                    ones_t[0:1, 0:DV],
                    srow[:, 512 * n : 512 * (n + 1)],
                    start=True,
                    stop=True,
                )
            rec = apool.tile([DV, SQ], F32R)
            nc.vector.reciprocal(rec[:], pb[:])
            oT_n = apool.tile([DV, SQ], F32R)
            nc.vector.tensor_mul(oT_n[:], oT_s[:], rec[:])
            pout = tc.alloc_tile_pool(name="pout", bufs=2, space="PSUM")
            for i in range(QB):
                pf = pout.tile([128, D], F32, tag="psout")
                nc.tensor.matmul(
                    pf[:],
                    oT_n[:, 128 * i : 128 * (i + 1)],
                    wo_t[:],
                    start=True,
                    stop=True,
                )
                osb = wpool.tile([128, D], F32, tag="osb")
                nc.vector.tensor_copy(osb[:], pf[:])
                nc.sync.dma_start(d_out[128 * i : 128 * (i + 1), :], osb[:])
            pout.release()
            pacc.release()
    nc.compile()
    return nc


_NC_CACHE = None


def _get_nc():
    global _NC_CACHE
    if _NC_CACHE is None:
        _NC_CACHE = _build()
    return _NC_CACHE


def make_in_maps(query, key, Wq, bq, Wk, bk, Wv, bv, Wo, bo):
    query = np.asarray(query, dtype=np.float32)
    key = np.asarray(key, dtype=np.float32)
    Wq = np.asarray(Wq, dtype=np.float32)
    Wk = np.asarray(Wk, dtype=np.float32)
    Wv = np.asarray(Wv, dtype=np.float32)
    Wo = np.asarray(Wo, dtype=np.float32)
    bq = np.asarray(bq, dtype=np.float32)
    bk = np.asarray(bk, dtype=np.float32)

    wo_r = np.ascontiguousarray(Wo.reshape(H, DV, D).sum(axis=0))  # [DV, D]
    ones = np.ones((128, 65), np.float32)
    ident = np.eye(DV, dtype=np.float32)
    tri = np.triu(np.ones((128, 128), np.float32))  # valid: ks <= qs

    in_maps = []
    for c in range(N_CORES):
        b, p = divmod(c, 2)
        blocks = BLOCKS[p]
        rows = np.concatenate(
            [np.arange(128 * g, 128 * (g + 1)) for g in blocks]
        )
        qT = np.ascontiguousarray(query[b][rows].T)  # [D, SQ]
        kT = np.ascontiguousarray(key[b].T)  # [D, S]
        bm = np.empty((NB, 128, 128), np.float32)
        for j in range(NB):
            g = blocks[QB - WIDTHS[j]]
            if g == j:
                bm[j] = tri
            elif g > j:
                bm[j] = 1.0
            else:
                bm[j] = 0.0
        in_maps.append(
            {
                "qT": qT,
                "kT": kT,
                "wq": Wq,
                "wk": Wk,
                "wv": Wv,
                "wo": wo_r,
                "bq": bq.reshape(DK, 1),
                "bk": bk.reshape(DK, 1),
                "bm": bm,
                "ones": ones,
                "ident": ident,
            }
        )
    return in_maps


def gather_output(results, Wv_bias_term):
    """results: list of per-core {'out': [SQ, D]}; adds host-folded bias."""
    out = np.empty((B, S, D), np.float32)
    for c in range(N_CORES):
        b, p = divmod(c, 2)
        blocks = BLOCKS[p]
        co = results[c]["out"]
        for bp, g in enumerate(blocks):
            out[b, 128 * g : 128 * (g + 1), :] = co[128 * bp : 128 * (bp + 1), :]
    out += Wv_bias_term
    return out


def kernel(query, key, value, Wq, bq, Wk, bk, Wv, bv, Wo, bo):
    from concourse import bass_utils

    nc = _get_nc()
    in_maps = make_in_maps(query, key, Wq, bq, Wk, bk, Wv, bv, Wo, bo)
    res = bass_utils.run_bass_kernel_spmd(
        nc, in_maps, core_ids=list(range(N_CORES))
    )
    Wo = np.asarray(Wo, dtype=np.float32)
    wo_r = Wo.reshape(H, DV, D).sum(axis=0)
    bias_term = np.asarray(bv, np.float32) @ wo_r + np.asarray(bo, np.float32)
    return gather_output(res.results, bias_term.astype(np.float32))


# revision 29
# speedup vs baseline: 32561.6895x; 32561.6895x over previous
"""Trainium2 Bass kernel for nn_MultiHeadAttention_19396072309379.

Module math (per reference): all H=8 heads identical; V projected from `key`;
causal mask; softmax; concat of identical heads @ Wo  ==  o @ (sum of Wo row
blocks).  Computed as single-head attention with a reduced Wo.

Sharding: 8 cores = 4 batches x 2 "parity" halves.  Each core owns 8 of the 16
query blocks (128 rows each) of one batch, paired {i, 15-i} so causal work is
balanced (68 block-pairs per core).  Both parities run the SAME program: the
attention loop uses unified per-key-block suffix widths (max over parities) and
a per-core mask input resolves the diagonal/extra-block difference as data.

On-chip layout is fully transposed ("T" = [feature, seq]): scoresT[ks, qs] =
kT_proj_blk.T @ qT_proj; exp via ACT (scale=1/sqrt(DK) fused); P stays
transposed so PV needs no P transpose: oT = v1.T @ expT where v1 = [v | 1] --
the ones column accumulates the softmax denominators for free (row 64).
Normalization is folded in after a K=1 broadcast matmul of the sums row.
All matmuls run fp32r (full-rate fp32 mode, ~1.6e-4 rel err).

Pipelining: key/value flow in 512-column chunks -- DMA chunk n -> k/v proj ->
v transpose -> attention key-blocks 4n..4n+3, so compute hides under the input
DMA.  The PV accumulator is split into two PSUM banks (query blocks 0-3 / 4-7);
the first half finalizes at j=7, so its normalization + output projection +
store overlap the second half of the attention loop.
"""

import numpy as np

B, S, D, H, DK, DV = 4, 2048, 512, 8, 64, 64
NB = S // 128  # 16 key/query blocks per batch
QB = 8  # query blocks per core
SQ = QB * 128  # 1024 query rows per core
N_CORES = 8

# per-parity query block sets (pairs {i, 15-i} -> equal causal work 68)
BLOCKS = {
    0: [0, 2, 4, 6, 9, 11, 13, 15],
    1: [1, 3, 5, 7, 8, 10, 12, 14],
}
# unified suffix width (in 128-blocks) for key-block j = max over parities of
# count of local query blocks with global index >= j
WIDTHS = [
    max(sum(1 for g in BLOCKS[p] if g >= j) for p in (0, 1)) for j in range(NB)
]


def _build(reps=1):
    import concourse.mybir as mybir
    import concourse.tile as tile
    from concourse import bacc

    F32 = mybir.dt.float32
    F32R = mybir.dt.float32r
    U8 = mybir.dt.uint8
    AF = mybir.ActivationFunctionType

    nc = bacc.Bacc("TRN2", target_bir_lowering=False, debug=False, num_devices=N_CORES)
    F16 = mybir.dt.float16
    d_qT = nc.dram_tensor("qT", [D, SQ], F16, kind="ExternalInput").ap()
    d_kT = nc.dram_tensor("kT", [D, S], F16, kind="ExternalInput").ap()
    d_wqkv = nc.dram_tensor("wqkv", [D, 3 * DK], F16, kind="ExternalInput").ap()
    d_c64 = nc.dram_tensor("c64", [DV, D + 2 + 2 * DV], F32, kind="ExternalInput").ap()
    d_bm = nc.dram_tensor("bm", [128, NB + 1, 128], U8, kind="ExternalInput").ap()
    d_out = nc.dram_tensor("out", [SQ, D], F32, kind="ExternalOutput").ap()

    for _ in range(reps):
        _emit_body(
            nc, tile, mybir, F32, F32R, AF,
            d_qT, d_kT, d_wqkv, d_c64, d_bm, d_out,
        )
    nc.compile()
    return nc


def _emit_body(nc, tile, mybir, F32, F32R, AF,
               d_qT, d_kT, d_wqkv, d_c64, d_bm, d_out):
    F16 = mybir.dt.float16
    with (
        tile.TileContext(nc) as tc,
        nc.allow_low_precision(reason="fp32r attention kernel"),
    ):
        with (
            tc.tile_pool(name="const", bufs=1) as cpool,
            tc.tile_pool(name="acts", bufs=1) as apool,
            tc.tile_pool(name="work", bufs=3) as wpool,
            tc.tile_pool(name="psmall", bufs=2, space="PSUM") as psm,
            tc.tile_pool(name="pscore", bufs=3, space="PSUM") as psc,
            tc.tile_pool(name="pacc", bufs=1, space="PSUM") as pacc,
        ):
            # ---- weights first (small), then query chunk 0: the q-proj
            # chain gates everything and HWDGE transfers serialize in
            # emission order ----
            # wqkv_t[:, c, 0:64]=Wq, 64:128=Wk, 128:192=Wv (d-chunk c)
            wqkv_t = cpool.tile([128, 4, 3 * DK], F16)
            nc.scalar.dma_start(
                wqkv_t[:], d_wqkv.rearrange("(c p) k -> p c k", p=128)
            )
            qT_act = apool.tile([128, 4, SQ], F16)
            qT_r0 = d_qT.rearrange("(c p) s -> p c s", p=128)
            nc.sync.dma_start(qT_act[:, :, 0:512], qT_r0[:, :, 0:512])
            # uint8 mask (with extra all-ones plane), cast to f32 in-flight
            # on the SWDGE ring, parallel to HWDGE
            bm_t = cpool.tile([128, NB + 1, 128], F32)
            nc.gpsimd.dma_start(bm_t[:], d_bm[:])
            # c64_t: [64, 0:512]=Wo_r, 512=bq, 513=bk, 514:578=id, 578:642=1s
            c64_t = cpool.tile([DV, D + 2 + 2 * DV], F32R)
            nc.scalar.dma_start(c64_t[:], d_c64.bitcast(F32R))
            wq_t = wqkv_t[:, :, 0:DK]
            wk_t = wqkv_t[:, :, DK : 2 * DK]
            wv_t = wqkv_t[:, :, 2 * DK : 3 * DK]
            wo_t = c64_t[:, 0:D]
            bq_t = c64_t[:, D : D + 1].bitcast(F32)
            bk_t = c64_t[:, D + 1 : D + 2].bitcast(F32)
            id_t = c64_t[:, D + 2 : D + 2 + DV].bitcast(F32)
            ones_row = c64_t[0:1, D + 2 + DV : D + 2 + 2 * DV]

            # ---- persistent SBUF tensors ----
            kT_act = apool.tile([128, 4, S], F16)
            qT_proj = apool.tile([DK, SQ], F32R)
            kT_proj = apool.tile([DK, S], F32R)
            vT_proj = apool.tile([DV, S], F32)
            v1 = apool.tile([128, NB, DV + 1], F32R)
            # accumulator regions: [lo, hi, last_j]; each its own PSUM bank
            PO_R = [(0, 512, 7), (512, 768, 11), (768, 1024, 15)]
            po = [
                pacc.tile([DV + 1, hi - lo], F32, name=f"po{q}")
                for q, (lo, hi, _) in enumerate(PO_R)
            ]

            qT_r = d_qT.rearrange("(c p) s -> p c s", p=128)
            kT_r = d_kT.rearrange("(c p) s -> p c s", p=128)

            # ---- remaining input DMAs up front (few, large) ----
            nc.sync.dma_start(qT_act[:, :, 512:1024], qT_r[:, :, 512:1024])
            for n in range(S // 512):
                sl = slice(512 * n, 512 * (n + 1))
                nc.sync.dma_start(kT_act[:, :, sl], kT_r[:, :, sl])
            # ones column of every v1 block in one shot (from the mask's
            # all-ones plane)
            nc.vector.tensor_copy(
                v1[:, :, DV : DV + 1].rearrange("p j o -> p (j o)"),
                bm_t[:, NB, 0:NB].bitcast(F32R),
            )

            def qproj(n):
                sl = slice(512 * n, 512 * (n + 1))
                ps = psm.tile([DK, 512], F32, tag="pp", name="psq")
                for c in range(4):
                    nc.tensor.matmul(
                        ps[:], wq_t[:, c, :], qT_act[:, c, sl],
                        start=(c == 0), stop=(c == 3),
                    )
                nc.vector.tensor_scalar_add(qT_proj[:, sl], ps[:], bq_t[:])

            def kvproj(n):
                sl = slice(512 * n, 512 * (n + 1))
                ps = psm.tile([DK, 512], F32, tag="pp", name="psk")
                for c in range(4):
                    nc.tensor.matmul(
                        ps[:], wk_t[:, c, :], kT_act[:, c, sl],
                        start=(c == 0), stop=(c == 3),
                    )
                nc.vector.tensor_scalar_add(kT_proj[:, sl], ps[:], bk_t[:])
                ps = psm.tile([DV, 512], F32, tag="pp", name="psv")
                for c in range(4):
                    nc.tensor.matmul(
                        ps[:], wv_t[:, c, :], kT_act[:, c, sl],
                        start=(c == 0), stop=(c == 3),
                    )
                nc.vector.tensor_copy(vT_proj[:, sl], ps[:])

            def transpose_v(j):
                # v1[j][:, 0:DV] = v block via PE transpose of vT columns
                pt = psm.tile([128, DV], F32, tag="pp", name="pt")
                nc.tensor.transpose(
                    pt[:], vT_proj[:, 128 * j : 128 * (j + 1)], id_t[:]
                )
                nc.vector.tensor_copy(v1[:, j, 0:DV], pt[:])

            exps = {}

            def scores_exp(j):
                wblk = WIDTHS[j]
                c0 = 128 * (QB - wblk)
                cols = 128 * wblk
                expT = wpool.tile([128, 1024], F32R, tag="expT", bufs=6)
                for m in range((cols + 511) // 512):
                    nsz = min(512, cols - 512 * m)
                    ps_s = psc.tile([128, 512], F32, tag="pss")
                    nc.tensor.matmul(
                        ps_s[:, 0:nsz],
                        kT_proj[:, 128 * j : 128 * (j + 1)],
                        qT_proj[:, c0 + 512 * m : c0 + 512 * m + nsz],
                        start=True, stop=True,
                    )
                    nc.scalar.activation(
                        expT[:, 512 * m : 512 * m + nsz], ps_s[:, 0:nsz],
                        AF.Exp, bias=0.0, scale=0.125,
                    )
                nc.vector.tensor_mul(
                    expT[:, 0:128], expT[:, 0:128],
                    bm_t[:, j, :].bitcast(F32R),
                )
                exps[j] = expT

            def emit_pv(j, part):
                """part 0: columns past the masked first block (independent of
                the mask op); part 1: the masked first 128 columns; part 2:
                the full range (used for j == 0, whose start=True clears the
                whole PSUM bank and therefore must be a single first write)."""
                wblk = WIDTHS[j]
                c0 = 128 * (QB - wblk)
                lo0, hi0 = (c0 + 128, SQ) if part == 0 else (c0, c0 + 128)
                if part == 2:
                    lo0, hi0 = c0, SQ
                for q, (rlo, rhi, lastj) in enumerate(PO_R):
                    lo = max(lo0, rlo)
                    hi = min(hi0, rhi)
                    if lo >= hi:
                        continue
                    nc.tensor.matmul(
                        po[q][:, lo - rlo : hi - rlo],
                        v1[:, j, :],
                        exps[j][:, lo - c0 : hi - c0],
                        start=(j == 0),
                        stop=(j == lastj and part != 0),
                        skip_group_check=True,
                    )

            # staged epilogue for accumulator region q
            ep_state = {}

            def ep_a(q):
                p = po[q]
                w = PO_R[q][1] - PO_R[q][0]
                oT_s = apool.tile([DV, 512], F32R, name=f"oTs{q}", tag=f"oTs{q}")
                nc.scalar.copy(oT_s[:, 0:w], p[0:DV, :])
                srow = apool.tile([1, 512], F32R, name=f"srow{q}", tag=f"srow{q}")
                nc.vector.tensor_copy(srow[:, 0:w], p[DV : DV + 1, :])
                pb = psm.tile([DV, 512], F32, tag="pp", name=f"pb{q}")
                nc.tensor.matmul(
                    pb[:, 0:w], ones_row, srow[:, 0:w],
                    start=True, stop=True,
                )
                ep_state[q] = (oT_s, pb)

            def ep_b(q):
                oT_s, pb = ep_state[q]
                w = PO_R[q][1] - PO_R[q][0]
                rec = apool.tile([DV, 512], F32R, name=f"rec{q}", tag=f"rec{q}")
                nc.vector.reciprocal(rec[:, 0:w], pb[:, 0:w])
                oT_n = apool.tile([DV, 512], F32R, name=f"oTn{q}", tag=f"oTn{q}")
                nc.vector.tensor_mul(oT_n[:, 0:w], oT_s[:, 0:w], rec[:, 0:w])
                ep_state[q] = oT_n

            def ep_c(q, i):
                oT_n = ep_state[q]
                pf = psm.tile([128, D], F32, tag="pp", name=f"pf{q}")
                nc.tensor.matmul(
                    pf[:], oT_n[:, 128 * i : 128 * (i + 1)], wo_t[:],
                    start=True, stop=True,
                )
                osb = wpool.tile([128, D], F32, tag="osb", name=f"osb{q}")
                if i % 2 == 0:
                    nc.vector.tensor_copy(osb[:], pf[:])
                else:
                    nc.scalar.copy(osb[:], pf[:])
                qb = PO_R[q][0] // 128 + i
                eng = nc.sync if i % 2 == 0 else nc.scalar
                eng.dma_start(d_out[128 * qb : 128 * (qb + 1), :], osb[:])

            # ---- emission schedule: projections lead their consumer group;
            # PV trails exp (bulk by 1, masked block by 2); epilogue(0) is
            # spread across iterations 9..14 so the in-order PE never camps
            # behind its serial ACT->DVE chain. ----
            EP0 = {9: lambda: ep_a(0), 10: lambda: ep_b(0),
                   11: lambda: ep_c(0, 0), 12: lambda: ep_c(0, 1),
                   13: lambda: (ep_c(0, 2), ep_a(1)),
                   14: lambda: (ep_c(0, 3), ep_b(1)),
                   15: lambda: ep_c(1, 0)}
            qproj(0)
            qproj(1)
            kvproj(0)
            kvproj(1)
            for j in range(4):
                transpose_v(j)
            for j in range(NB):
                if j == 2:
                    for jj in range(4, 8):
                        transpose_v(jj)
                if j == 4:
                    kvproj(2)
                if j == 6:
                    for jj in range(8, 12):
                        transpose_v(jj)
                if j == 8:
                    kvproj(3)
                if j == 10:
                    for jj in range(12, NB):
                        transpose_v(jj)
                scores_exp(j)
                if j == 1:
                    emit_pv(0, 2)  # j=0 unsplit: single start=True per bank
                elif j >= 2:
                    emit_pv(j - 1, 0)
                if j >= 3:
                    emit_pv(j - 2, 1)
                if j in EP0:
                    EP0[j]()
            emit_pv(NB - 1, 0)
            emit_pv(NB - 2, 1)
            ep_c(1, 1)
            emit_pv(NB - 1, 1)
            ep_a(2)
            ep_b(2)
            ep_c(2, 0)
            ep_c(2, 1)


_NC_CACHE = None


def _get_nc():
    global _NC_CACHE
    if _NC_CACHE is None:
        _NC_CACHE = _build()
    return _NC_CACHE


def make_in_maps(query, key, Wq, bq, Wk, bk, Wv, bv, Wo, bo):
    query = np.asarray(query, dtype=np.float32)
    key = np.asarray(key, dtype=np.float32)
    Wq = np.asarray(Wq, dtype=np.float32)
    Wk = np.asarray(Wk, dtype=np.float32)
    Wv = np.asarray(Wv, dtype=np.float32)
    Wo = np.asarray(Wo, dtype=np.float32)
    bq = np.asarray(bq, dtype=np.float32)
    bk = np.asarray(bk, dtype=np.float32)

    wo_r = np.ascontiguousarray(Wo.reshape(H, DV, D).sum(axis=0))  # [DV, D]
    wqkv = np.concatenate([Wq, Wk, Wv], axis=1).astype(np.float16)  # [D, 192]
    c64 = np.concatenate(
        [wo_r, bq.reshape(DV, 1), bk.reshape(DV, 1),
         np.eye(DV, dtype=np.float32), np.ones((DV, DV), np.float32)],
        axis=1,
    )  # [64, 642]
    tri = np.triu(np.ones((128, 128), np.uint8))  # valid: ks <= qs

    in_maps = []
    for c in range(N_CORES):
        b, p = divmod(c, 2)
        blocks = BLOCKS[p]
        rows = np.concatenate(
            [np.arange(128 * g, 128 * (g + 1)) for g in blocks]
        )
        qT = np.ascontiguousarray(query[b][rows].T).astype(np.float16)
        kT = np.ascontiguousarray(key[b].T).astype(np.float16)
        bm = np.empty((NB + 1, 128, 128), np.uint8)
        bm[NB] = 1
        for j in range(NB):
            g = blocks[QB - WIDTHS[j]]
            if g == j:
                bm[j] = tri
            elif g > j:
                bm[j] = 1
            else:
                bm[j] = 0
        bm = np.ascontiguousarray(bm.transpose(1, 0, 2))  # [128, NB, 128]
        in_maps.append(
            {"qT": qT, "kT": kT, "wqkv": wqkv, "c64": c64, "bm": bm}
        )
    return in_maps


def gather_output(results, bias_term):
    """results: list of per-core {'out': [SQ, D]}; adds host-folded bias."""
    out = np.empty((B, S, D), np.float32)
    for c in range(N_CORES):
        b, p = divmod(c, 2)
        blocks = BLOCKS[p]
        co = results[c]["out"]
        for bp, g in enumerate(blocks):
            out[b, 128 * g : 128 * (g + 1), :] = co[128 * bp : 128 * (bp + 1), :]
    out += bias_term
    return out


def kernel(query, key, value, Wq, bq, Wk, bk, Wv, bv, Wo, bo):
    from concourse import bass_utils

    nc = _get_nc()
    in_maps = make_in_maps(query, key, Wq, bq, Wk, bk, Wv, bv, Wo, bo)
    res = bass_utils.run_bass_kernel_spmd(
        nc, in_maps, core_ids=list(range(N_CORES))
    )
    Wo = np.asarray(Wo, dtype=np.float32)
    wo_r = Wo.reshape(H, DV, D).sum(axis=0)
    bias_term = np.asarray(bv, np.float32) @ wo_r + np.asarray(bo, np.float32)
    return gather_output(res.results, bias_term.astype(np.float32))
